# revision 6
# baseline (speedup 1.0000x reference)
"""MoE routing kernel for Trainium2, 8 NeuronCores, token-parallel.

Problem (nn_Network_2121713845020):
  h = x @ W_in + b_in                        [N, D]
  probs = softmax(h @ W_gate); top-2 renormalized combine weights
  moe = sum_e combine[:, e] * (relu(h @ W1[e] + b1[e]) @ W2[e] + b2[e])
  out = moe @ W_head                         [N, OUT]

Strategy: shard tokens across 8 cores (N/8 = 2048 each); every core holds
all expert weights, computes routing on-device in fp32, compacts per-expert
token ids with gpsimd sparse_gather, gathers assigned token rows with
dma_gather (capacity C=640 vs expected 512), runs the expert FFN in bf16
with fp32 accumulation, scales by gathered combine weights and
dma_scatter_adds back, then applies the head. Device returns out^T per
core; the host transposes and concatenates. No collectives.
"""

import os
import sys

sys.path.insert(0, "/opt/trn_rl_repo")

from contextlib import ExitStack

import numpy as np
import ml_dtypes

import concourse.bacc as bacc
import concourse.bass as bass
import concourse.mybir as mybir
import concourse.tile as tile

f32 = mybir.dt.float32
bf16 = mybir.dt.bfloat16
i16 = mybir.dt.int16
u32 = mybir.dt.uint32
AF = mybir.ActivationFunctionType
ALU = mybir.AluOpType

N_CORES = 8

if os.environ.get("MOE_SMALL"):
    N, D, H, E, OUT, C = 4096, 512, 1024, 8, 512, 256
else:
    N, D, H, E, OUT, C = 16384, 1024, 4096, 8, 4096, 640

T = N // N_CORES            # tokens per core
TPAD = T + 128              # +sentinel row space
SENT = T                    # sentinel token id (garbage row)
KD = D // 128               # K-tiles over D
MH = H // 128               # M-tiles over H
HB = H // 1024              # H blocks of 1024 (8 m-tiles each)
C5 = C // 128               # compact-token tiles
NCH = T // 512              # N chunks of 512 over tokens
FSG = T // 16 + C // 16     # sparse_gather input free size


def build_program():
    nc = bacc.Bacc("TRN2", target_bir_lowering=False, debug=False,
                   num_devices=N_CORES)

    xT_d = nc.dram_tensor("xT", [D, T], f32, kind="ExternalInput")
    w_in_d = nc.dram_tensor("w_in", [D, D], bf16, kind="ExternalInput")
    b_in_d = nc.dram_tensor("b_in_c", [128, KD], f32, kind="ExternalInput")
    wg_eff_d = nc.dram_tensor("wg_eff", [D, E], f32, kind="ExternalInput")
    bg_eff_d = nc.dram_tensor("bg_eff", [E, 1], f32, kind="ExternalInput")
    w1_d = nc.dram_tensor("w1", [E, D, H], bf16, kind="ExternalInput")
    b1_d = nc.dram_tensor("b1_c", [E, 128, MH], f32, kind="ExternalInput")
    w2_d = nc.dram_tensor("w2", [E, H, D], bf16, kind="ExternalInput")
    b2_d = nc.dram_tensor("b2_r", [E, 128, D], bf16, kind="ExternalInput")
    w_head_d = nc.dram_tensor("w_head", [D, OUT], bf16, kind="ExternalInput")
    outT_d = nc.dram_tensor("outT", [OUT, T], f32, kind="ExternalOutput")

    h_dram = nc.dram_tensor("h_scr", [TPAD, D], bf16)
    wcomb_d = nc.dram_tensor("wcomb_scr", [TPAD, 128], f32)
    mid_d = nc.dram_tensor("mid_scr", [E, T], f32)
    moe_d = nc.dram_tensor("moe_scr", [TPAD, D], bf16)

    idb_np = np.eye(128, dtype=ml_dtypes.bfloat16)
    idb_d = nc.inline_tensor(np.ascontiguousarray(idb_np), name="id_bf")
    idf_np = np.eye(128, dtype=np.float32)
    idf_d = nc.inline_tensor(np.ascontiguousarray(idf_np), name="id_f32")
    iota8_np = np.tile(np.arange(T, dtype=np.float32)[None, :], (E, 1))
    iota8_d = nc.inline_tensor(np.ascontiguousarray(iota8_np), name="iota8")

    with tile.TileContext(nc) as tc, ExitStack() as octx:
        const = octx.enter_context(tc.tile_pool(name="const", bufs=1))
        idb = const.tile([128, 128], bf16, tag="idb")
        nc.sync.dma_start(out=idb[:], in_=idb_d[:])
        idf = const.tile([128, 128], f32, tag="idf")
        nc.sync.dma_start(out=idf[:], in_=idf_d[:])

        persist = octx.enter_context(tc.tile_pool(name="persist", bufs=1))
        lg = persist.tile([E, T], f32, tag="lg")
        hT_ctx = ExitStack()
        hT_pool = hT_ctx.enter_context(tc.tile_pool(name="hT_pool", bufs=1))
        hT = hT_pool.tile([128, KD, T], bf16, tag="hT")

        # ---------------- P1: h^T (bf16) and fp32 gating logits ----------------
        with tc.tile_pool(name="p1sb", bufs=1) as p1c, \
             tc.tile_pool(name="p1ps", bufs=3, space="PSUM") as p1ps, \
             tc.tile_pool(name="p1lg", bufs=1, space="PSUM") as p1lg:
            xT_f = p1c.tile([128, KD, T], f32, tag="xTf")
            nc.sync.dma_start(
                out=xT_f[:], in_=xT_d.ap().rearrange("(k p) t -> p k t", p=128))
            xT_t = p1c.tile([128, KD, T], bf16, tag="xT")
            for k in range(KD):
                nc.vector.tensor_copy(xT_t[:, k, :], xT_f[:, k, :])
            w_in_t = p1c.tile([128, KD, D], bf16, tag="w_in")
            nc.sync.dma_start(
                out=w_in_t[:], in_=w_in_d.ap().rearrange("(k p) m -> p k m", p=128))
            wg_t = p1c.tile([128, KD, E], f32, tag="wg_eff")
            nc.sync.dma_start(
                out=wg_t[:], in_=wg_eff_d.ap().rearrange("(k p) e -> p k e", p=128))
            bg_t = p1c.tile([E, 1], f32, tag="bg_eff")
            nc.sync.dma_start(out=bg_t[:], in_=bg_eff_d[:])
            b_in_t = p1c.tile([128, KD], f32, tag="b_in")
            nc.sync.dma_start(out=b_in_t[:], in_=b_in_d[:])

            # fp32 gating logits: lg = x @ (W_in @ W_gate) + b_in @ W_gate
            lg_ps = p1lg.tile([E, T], f32, tag="lg_ps")
            for ch in range(T // 512):
                for k in range(KD):
                    nc.tensor.matmul(
                        lg_ps[:, ch * 512:(ch + 1) * 512],
                        wg_t[:, k, :],
                        xT_f[:, k, ch * 512:(ch + 1) * 512],
                        start=(k == 0), stop=(k == KD - 1))
            nc.vector.tensor_scalar(lg[:], lg_ps[:], bg_t[:], None, ALU.add)

            # h^T in bf16
            for m in range(KD):
                for ch in range(T // 512):
                    hps = p1ps.tile([128, 512], f32, tag="hps")
                    for k in range(KD):
                        nc.tensor.matmul(
                            hps[:],
                            w_in_t[:, k, m * 128:(m + 1) * 128],
                            xT_t[:, k, ch * 512:(ch + 1) * 512],
                            start=(k == 0), stop=(k == KD - 1))
                    nc.scalar.activation(
                        hT[:, m, ch * 512:(ch + 1) * 512], hps[:],
                        AF.Identity, bias=b_in_t[:, m:m + 1])

        # ---------------- P2: routing (fp32) ----------------
        with tc.tile_pool(name="p2sb", bufs=1) as p2, \
             tc.tile_pool(name="p2ps", bufs=2, space="PSUM") as p2ps:
            m1 = p2.tile([32, T], f32, tag="m1")
            m2 = p2.tile([32, T], f32, tag="m2")
            s1 = p2.tile([32, T], f32, tag="s1")
            s2 = p2.tile([32, T], f32, tag="s2")
            tmin = p2.tile([8, T], f32, tag="tmin")
            nc.vector.memset(m1[:], -1e30)
            nc.vector.memset(m2[:], -1e30)
            nc.vector.tensor_copy(m1[0:8, :], lg[:])
            for st in (4, 2, 1):
                shuf = [(i ^ st) for i in range(32)]
                nc.vector.stream_shuffle(s1[:], m1[:], shuf)
                nc.vector.stream_shuffle(s2[:], m2[:], shuf)
                nc.vector.tensor_tensor(tmin[:], m1[0:8, :], s1[0:8, :], ALU.min)
                nc.vector.tensor_tensor(m1[0:8, :], m1[0:8, :], s1[0:8, :], ALU.max)
                nc.vector.tensor_tensor(m2[0:8, :], m2[0:8, :], s2[0:8, :], ALU.max)
                nc.vector.tensor_tensor(m2[0:8, :], m2[0:8, :], tmin[:], ALU.max)

            lsub = p2.tile([E, T], f32, tag="lsub")
            nc.vector.tensor_sub(lsub[:], lg[:], m1[0:8, :])
            p_t = p2.tile([E, T], f32, tag="p")
            nc.scalar.activation(p_t[:], lsub[:], AF.Exp)
            mask = p2.tile([E, T], f32, tag="mask")
            nc.vector.tensor_tensor(mask[:], lg[:], m2[0:8, :], ALU.is_ge)
            dgap = p2.tile([E, T], f32, tag="dgap")
            nc.vector.tensor_sub(dgap[:], m2[0:8, :], m1[0:8, :])
            dexp = p2.tile([E, T], f32, tag="dexp")
            nc.scalar.activation(dexp[:], dgap[:], AF.Exp)
            nc.vector.tensor_scalar(dexp[:], dexp[:], 1.0, None, ALU.add)
            rec = p2.tile([E, T], f32, tag="rec")
            nc.vector.reciprocal(rec[:], dexp[:])
            comb = p2.tile([E, T], f32, tag="comb")
            nc.vector.tensor_mul(comb[:], p_t[:], mask[:])
            nc.vector.tensor_mul(comb[:], comb[:], rec[:])

            iota8_t = p2.tile([E, T], f32, tag="iota8")
            nc.sync.dma_start(out=iota8_t[:], in_=iota8_d[:])
            mid = p2.tile([E, T], f32, tag="mid")
            nc.vector.tensor_scalar(mid[:], iota8_t[:], 1.0, None, ALU.add)
            nc.vector.tensor_mul(mid[:], mid[:], mask[:])
            nc.vector.tensor_scalar(mid[:], mid[:], 1.0, None, ALU.subtract)
            nc.sync.dma_start(out=mid_d[:], in_=mid[:])

            # combine -> wcomb rows [t, 0:8]; zero-fill sentinel rows
            zf = p2.tile([128, 128], f32, tag="zf")
            nc.vector.memset(zf[:], 0.0)
            nc.sync.dma_start(out=wcomb_d[T:TPAD, :], in_=zf[:TPAD - T, :])
            for g in range(T // 128):
                cps = p2ps.tile([128, E], f32, tag="cps")
                nc.tensor.transpose(
                    cps[:], comb[:, g * 128:(g + 1) * 128], idf[:E, :E])
                csb = p2.tile([128, E], f32, tag="csb")
                nc.vector.tensor_copy(csb[:], cps[:])
                nc.sync.dma_start(
                    out=wcomb_d[g * 128:(g + 1) * 128, 0:8], in_=csb[:])

        # ---------------- P3: h rows to DRAM (+ zero fills) ----------------
        with tc.tile_pool(name="p3sb", bufs=3) as p3, \
             tc.tile_pool(name="p3ps", bufs=4, space="PSUM") as p3ps:
            zh = p3.tile([128, D], bf16, tag="zh")
            nc.vector.memset(zh[:], 0.0)
            nc.sync.dma_start(out=h_dram[T:TPAD, :], in_=zh[:TPAD - T, :])
            for g in range(TPAD // 128):
                nc.sync.dma_start(
                    out=moe_d[g * 128:(g + 1) * 128, :], in_=zh[:])
            for g in range(T // 128):
                hrow = p3.tile([128, D], bf16, tag="hrow")
                for m in range(KD):
                    tps = p3ps.tile([128, 128], bf16, tag="tps")
                    nc.tensor.transpose(
                        tps[:], hT[:, m, g * 128:(g + 1) * 128], idb[:])
                    nc.vector.tensor_copy(hrow[:, m * 128:(m + 1) * 128], tps[:])
                nc.sync.dma_start(out=h_dram[g * 128:(g + 1) * 128, :], in_=hrow[:])

        hT_ctx.close()

        # ---------------- P4: expert FFNs on compacted tokens ----------------
        with tc.tile_pool(name="p4idx", bufs=2) as p4i, \
             tc.tile_pool(name="p4g", bufs=2) as p4g, \
             tc.tile_pool(name="p4w", bufs=2) as p4w, \
             tc.tile_pool(name="p4he", bufs=2) as p4he, \
             tc.tile_pool(name="p4y", bufs=2) as p4y, \
             tc.tile_pool(name="p4ps1", bufs=2, space="PSUM") as ps1, \
             tc.tile_pool(name="p4ps2", bufs=3, space="PSUM") as ps2:
            for e in range(E):
                sgin = p4i.tile([16, FSG], f32, tag="sgin")
                nc.sync.dma_start(
                    out=sgin[:, :T // 16],
                    in_=mid_d.ap()[e].rearrange("(f p) -> p f", p=16))
                nc.vector.memset(sgin[:, T // 16:], float(SENT))
                sgout = p4i.tile([16, FSG], f32, tag="sgout")
                nf = p4i.tile([1, 1], u32, tag="nf")
                nc.gpsimd.sparse_gather(sgout[:], sgin[:], num_found=nf[:])
                idx16 = p4i.tile([16, C // 16], i16, tag="idx16")
                nc.vector.tensor_copy(idx16[:], sgout[:, :C // 16])
                idxr = p4i.tile([128, C // 16], i16, tag="idxr")
                for q in range(8):
                    nc.sync.dma_start(
                        out=idxr[q * 16:(q + 1) * 16, :], in_=idx16[:])

                ghT = p4g.tile([128, KD, C], bf16, tag="ghT")
                nc.gpsimd.dma_gather(
                    ghT[:], h_dram[:], idxr[:], C, C, D, transpose=True)
                gw = p4g.tile([128, C5, 128], f32, tag="gw")
                nc.gpsimd.dma_gather(
                    gw[:], wcomb_d[:], idxr[:], C, C, 128, transpose=False)

                b1_t = p4i.tile([128, MH], f32, tag="b1")
                nc.sync.dma_start(out=b1_t[:], in_=b1_d[e])
                b2_t = p4i.tile([128, D], bf16, tag="b2")
                nc.sync.dma_start(out=b2_t[:], in_=b2_d[e])

                y_acc = p4y.tile([128, C5, D], f32, tag="y_acc")
                for hb in range(HB):
                    w1_blk = p4w.tile([128, KD, 1024], bf16, tag="w1_blk")
                    nc.sync.dma_start(
                        out=w1_blk[:],
                        in_=w1_d.ap()[e, :, hb * 1024:(hb + 1) * 1024]
                        .rearrange("(k p) m -> p k m", p=128))
                    w2_blk = p4w.tile([128, 8, D], bf16, tag="w2_blk")
                    nc.sync.dma_start(
                        out=w2_blk[:],
                        in_=w2_d.ap()[e, hb * 1024:(hb + 1) * 1024, :]
                        .rearrange("(k p) n -> p k n", p=128))

                    he_blk = p4he.tile([128, 8, C], bf16, tag="he_blk")
                    for m8 in range(8):
                        p1t = ps1.tile([128, C], f32, tag="p1t")
                        for ch0 in range(0, C, 512):
                            ch1 = min(ch0 + 512, C)
                            for k in range(KD):
                                nc.tensor.matmul(
                                    p1t[:, ch0:ch1],
                                    w1_blk[:, k, m8 * 128:(m8 + 1) * 128],
                                    ghT[:, k, ch0:ch1],
                                    start=(k == 0), stop=(k == KD - 1))
                        nc.scalar.activation(
                            he_blk[:, m8, :], p1t[:], AF.Relu,
                            bias=b1_t[:, hb * 8 + m8:hb * 8 + m8 + 1])

                    for c5 in range(C5):
                        for ch in range(D // 512):
                            p2t = ps2.tile([128, 512], f32, tag="p2t")
                            for k8 in range(8):
                                nc.tensor.matmul(
                                    p2t[:],
                                    he_blk[:, k8, c5 * 128:(c5 + 1) * 128],
                                    w2_blk[:, k8, ch * 512:(ch + 1) * 512],
                                    start=(k8 == 0), stop=(k8 == 7))
                            dst = y_acc[:, c5, ch * 512:(ch + 1) * 512]
                            if hb == 0:
                                nc.vector.tensor_copy(dst, p2t[:])
                            else:
                                nc.vector.tensor_add(dst, dst, p2t[:])

                ysb = p4y.tile([128, C5, D], bf16, tag="ysb")
                for c5 in range(C5):
                    nc.vector.tensor_add(
                        y_acc[:, c5, :], y_acc[:, c5, :], b2_t[:])
                    nc.vector.tensor_scalar(
                        ysb[:, c5, :], y_acc[:, c5, :],
                        gw[:, c5, e:e + 1], None, ALU.mult)
                nc.gpsimd.dma_scatter_add(moe_d[:], ysb[:], idxr[:], C, C, D)

        # ---------------- P5: moe reload + transpose ----------------
        late = octx.enter_context(tc.tile_pool(name="late", bufs=1))
        with tc.tile_pool(name="p5sb", bufs=3) as p5, \
             tc.tile_pool(name="p5ps", bufs=4, space="PSUM") as p5ps:
            moeT = late.tile([128, KD, T], bf16, tag="moeT")
            for g in range(T // 128):
                mrow = p5.tile([128, D], bf16, tag="mrow")
                nc.sync.dma_start(
                    out=mrow[:], in_=moe_d[g * 128:(g + 1) * 128, :])
                for m in range(KD):
                    tps = p5ps.tile([128, 128], bf16, tag="tps5")
                    nc.tensor.transpose(
                        tps[:], mrow[:, m * 128:(m + 1) * 128], idb[:])
                    nc.vector.tensor_copy(
                        moeT[:, m, g * 128:(g + 1) * 128], tps[:])

        # ---------------- P6: head ----------------
        with tc.tile_pool(name="p6w", bufs=2) as p6w, \
             tc.tile_pool(name="p6o", bufs=3) as p6o, \
             tc.tile_pool(name="p6ps", bufs=3, space="PSUM") as p6ps:
            OBLK = 1024 if OUT >= 1024 else OUT
            for mb in range(OUT // OBLK):
                wh_blk = p6w.tile([128, KD, OBLK], bf16, tag="wh_blk")
                nc.sync.dma_start(
                    out=wh_blk[:],
                    in_=w_head_d.ap()[:, mb * OBLK:(mb + 1) * OBLK]
                    .rearrange("(k p) m -> p k m", p=128))
                for m8 in range(OBLK // 128):
                    orow = p6o.tile([128, T], f32, tag="orow")
                    for ch in range(T // 512):
                        pht = p6ps.tile([128, 512], f32, tag="pht")
                        for k in range(KD):
                            nc.tensor.matmul(
                                pht[:],
                                wh_blk[:, k, m8 * 128:(m8 + 1) * 128],
                                moeT[:, k, ch * 512:(ch + 1) * 512],
                                start=(k == 0), stop=(k == KD - 1))
                        nc.vector.tensor_copy(
                            orow[:, ch * 512:(ch + 1) * 512], pht[:])
                    r0 = mb * OBLK + m8 * 128
                    nc.sync.dma_start(out=outT_d[r0:r0 + 128, :], in_=orow[:])

    nc.compile()
    return nc


_NC_CACHE = None


def get_program():
    global _NC_CACHE
    if _NC_CACHE is None:
        _NC_CACHE = build_program()
    return _NC_CACHE


def prep_in_maps(x, W_in, b_in, W_gate, W1, b1, W2, b2, W_head):
    bf = ml_dtypes.bfloat16
    w_in_h = np.ascontiguousarray(W_in.astype(bf))
    b_in_h = np.ascontiguousarray(b_in.astype(np.float32).reshape(KD, 128).T)
    wg_eff_h = np.ascontiguousarray(
        W_in.astype(np.float32) @ W_gate.astype(np.float32))
    bg_eff_h = np.ascontiguousarray(
        (b_in.astype(np.float32) @ W_gate.astype(np.float32)).reshape(E, 1))
    w1_h = np.ascontiguousarray(W1.astype(bf))
    b1_h = np.ascontiguousarray(
        np.transpose(b1.astype(np.float32).reshape(E, MH, 128), (0, 2, 1)))
    w2_h = np.ascontiguousarray(W2.astype(bf))
    b2_h = np.ascontiguousarray(
        np.broadcast_to(b2.astype(bf)[:, None, :], (E, 128, D)))
    w_head_h = np.ascontiguousarray(W_head.astype(bf))
    xT = np.ascontiguousarray(x.astype(np.float32).T)

    in_maps = []
    for c in range(N_CORES):
        in_maps.append({
            "xT": np.ascontiguousarray(xT[:, c * T:(c + 1) * T]),
            "w_in": w_in_h,
            "b_in_c": b_in_h,
            "wg_eff": wg_eff_h,
            "bg_eff": bg_eff_h,
            "w1": w1_h,
            "b1_c": b1_h,
            "w2": w2_h,
            "b2_r": b2_h,
            "w_head": w_head_h,
        })

    return in_maps


def kernel(**inputs):
    from concourse.bass_utils import run_bass_kernel_spmd

    in_maps = prep_in_maps(**inputs)
    nc = get_program()
    res = run_bass_kernel_spmd(nc, in_maps, list(range(N_CORES)))
    out = np.empty((N, OUT), dtype=np.float32)
    for c in range(N_CORES):
        out[c * T:(c + 1) * T, :] = res.results[c]["outT"].T
    return out


# revision 11
# speedup vs baseline: 1.0306x; 1.0306x over previous
"""MoE routing kernel for Trainium2, 8 NeuronCores, token-parallel.

Problem (nn_Network_2121713845020):
  h = x @ W_in + b_in                        [N, D]
  probs = softmax(h @ W_gate); top-2 renormalized combine weights
  moe = sum_e combine[:, e] * (relu(h @ W1[e] + b1[e]) @ W2[e] + b2[e])
  out = moe @ W_head                         [N, OUT]

Strategy: shard tokens across 8 cores (N/8 = 2048 each); every core holds
all expert weights, computes routing on-device in fp32, compacts per-expert
token ids with gpsimd sparse_gather, gathers assigned token rows with
dma_gather (capacity C=640 vs expected 512), runs the expert FFN in bf16
with fp32 accumulation, scales by gathered combine weights and
dma_scatter_adds back, then applies the head. Device returns out^T per
core; the host transposes and concatenates. No collectives.
"""

import os
import sys

sys.path.insert(0, "/opt/trn_rl_repo")

from contextlib import ExitStack

import numpy as np
import ml_dtypes

import concourse.bacc as bacc
import concourse.bass as bass
import concourse.mybir as mybir
import concourse.tile as tile

f32 = mybir.dt.float32
bf16 = mybir.dt.bfloat16
i16 = mybir.dt.int16
u32 = mybir.dt.uint32
AF = mybir.ActivationFunctionType
ALU = mybir.AluOpType

N_CORES = 8

if os.environ.get("MOE_SMALL"):
    N, D, H, E, OUT, C = 4096, 512, 1024, 8, 512, 256
else:
    N, D, H, E, OUT, C = 16384, 1024, 4096, 8, 4096, 640

T = N // N_CORES            # tokens per core
TPAD = T + 128              # +sentinel row space
SENT = T                    # sentinel token id (garbage row)
KD = D // 128               # K-tiles over D
MH = H // 128               # M-tiles over H
HB = H // 1024              # H blocks of 1024 (8 m-tiles each)
C5 = C // 128               # compact-token tiles
NCH = T // 512              # N chunks of 512 over tokens
FSG = T // 16 + C // 16     # sparse_gather input free size


def build_program():
    nc = bacc.Bacc("TRN2", target_bir_lowering=False, debug=False,
                   num_devices=N_CORES)

    xT_d = nc.dram_tensor("xT", [D, T], f32, kind="ExternalInput")
    w_in_d = nc.dram_tensor("w_in", [D, D], bf16, kind="ExternalInput")
    b_in_d = nc.dram_tensor("b_in_bc", [128, D], f32, kind="ExternalInput")
    wg_eff_d = nc.dram_tensor("wg_eff", [D, E], f32, kind="ExternalInput")
    bg_eff_d = nc.dram_tensor("bg_eff", [E, 1], f32, kind="ExternalInput")
    w1_d = nc.dram_tensor("w1", [E, D, H], bf16, kind="ExternalInput")
    b1_d = nc.dram_tensor("b1_c", [E, 128, MH], f32, kind="ExternalInput")
    w2_d = nc.dram_tensor("w2", [E, H, D], bf16, kind="ExternalInput")
    b2_d = nc.dram_tensor("b2_r", [E, 128, D], bf16, kind="ExternalInput")
    w_head_d = nc.dram_tensor("w_head", [D, OUT], bf16, kind="ExternalInput")
    outT_d = nc.dram_tensor("outT", [OUT, T], f32, kind="ExternalOutput")

    h_dram = nc.dram_tensor("h_scr", [TPAD, D], bf16)
    wcomb_d = nc.dram_tensor("wcomb_scr", [TPAD, 128], f32)
    mid_d = nc.dram_tensor("mid_scr", [E, T], f32)
    moe_d = nc.dram_tensor("moe_scr", [TPAD, D], bf16)

    idb_np = np.eye(128, dtype=ml_dtypes.bfloat16)
    idb_d = nc.inline_tensor(np.ascontiguousarray(idb_np), name="id_bf")
    idf_np = np.eye(128, dtype=np.float32)
    idf_d = nc.inline_tensor(np.ascontiguousarray(idf_np), name="id_f32")
    iota8_np = np.tile(np.arange(T, dtype=np.float32)[None, :], (E, 1))
    iota8_d = nc.inline_tensor(np.ascontiguousarray(iota8_np), name="iota8")
    idx_id_np = np.zeros((128, T // 16), dtype=np.int16)
    for j in range(T):
        for q in range(8):
            idx_id_np[q * 16 + j % 16, j // 16] = j
    idx_id_d = nc.inline_tensor(np.ascontiguousarray(idx_id_np), name="idx_id")

    with tile.TileContext(nc) as tc, ExitStack() as octx:
        const = octx.enter_context(tc.tile_pool(name="const", bufs=1))
        idb = const.tile([128, 128], bf16, tag="idb")
        nc.sync.dma_start(out=idb[:], in_=idb_d[:])
        idf = const.tile([128, 128], f32, tag="idf")
        nc.sync.dma_start(out=idf[:], in_=idf_d[:])

        persist = octx.enter_context(tc.tile_pool(name="persist", bufs=1))
        lg = persist.tile([E, T], f32, tag="lg")
        idxr_all = persist.tile([128, E, C // 16], i16, tag="idxr_all")

        # ---------------- P1: h^T (bf16) and fp32 gating logits ----------------
        with tc.tile_pool(name="p1sb", bufs=1) as p1c, \
             tc.tile_pool(name="p1r", bufs=4) as p1r, \
             tc.tile_pool(name="p1ps", bufs=3, space="PSUM") as p1ps, \
             tc.tile_pool(name="p1lg", bufs=1, space="PSUM") as p1lg:
            xT_f = p1c.tile([128, KD, T], f32, tag="xTf")
            xT_t = p1c.tile([128, KD, T], bf16, tag="xT")
            xr = xT_d.ap().rearrange("(k p) t -> p k t", p=128)
            for k in range(KD):
                nc.sync.dma_start(out=xT_f[:, k, :], in_=xr[:, k, :])
                nc.vector.tensor_copy(xT_t[:, k, :], xT_f[:, k, :])
            w_in_t = p1c.tile([128, KD, D], bf16, tag="w_in")
            wr = w_in_d.ap().rearrange("(k p) m -> p k m", p=128)
            for k in range(KD):
                nc.sync.dma_start(out=w_in_t[:, k, :], in_=wr[:, k, :])
            wg_t = p1c.tile([128, KD, E], f32, tag="wg_eff")
            nc.sync.dma_start(
                out=wg_t[:], in_=wg_eff_d.ap().rearrange("(k p) e -> p k e", p=128))
            bg_t = p1c.tile([E, 1], f32, tag="bg_eff")
            nc.sync.dma_start(out=bg_t[:], in_=bg_eff_d[:])
            b_in_t = p1c.tile([128, D], f32, tag="b_in")
            nc.sync.dma_start(out=b_in_t[:], in_=b_in_d[:])

            # fp32 gating logits: lg = x @ (W_in @ W_gate) + b_in @ W_gate
            lg_ps = p1lg.tile([E, T], f32, tag="lg_ps")
            for ch in range(T // 512):
                for k in range(KD):
                    nc.tensor.matmul(
                        lg_ps[:, ch * 512:(ch + 1) * 512],
                        wg_t[:, k, :],
                        xT_f[:, k, ch * 512:(ch + 1) * 512],
                        start=(k == 0), stop=(k == KD - 1))
            nc.vector.tensor_scalar(lg[:], lg_ps[:], bg_t[:], None, ALU.add)

            # h rows (token-major) in bf16, DMA'd straight to DRAM
            for g in range(T // 128):
                hrow = p1r.tile([128, D], bf16, tag="hrow")
                for ch in range(D // 512):
                    hps = p1ps.tile([128, 512], f32, tag="hps")
                    for k in range(KD):
                        nc.tensor.matmul(
                            hps[:],
                            xT_t[:, k, g * 128:(g + 1) * 128],
                            w_in_t[:, k, ch * 512:(ch + 1) * 512],
                            start=(k == 0), stop=(k == KD - 1))
                    nc.vector.tensor_add(
                        hrow[:, ch * 512:(ch + 1) * 512], hps[:],
                        b_in_t[:, ch * 512:(ch + 1) * 512])
                nc.sync.dma_start(
                    out=h_dram[g * 128:(g + 1) * 128, :], in_=hrow[:])

        # ------- P2+P3: routing (fp32) + h rows to DRAM (interleaved) -------
        with tc.tile_pool(name="p2sb", bufs=1) as p2, \
             tc.tile_pool(name="p2i", bufs=2) as p2i, \
             tc.tile_pool(name="p3sb", bufs=3) as p3, \
             tc.tile_pool(name="p2ps", bufs=2, space="PSUM") as p2ps:
            m1 = p2.tile([32, T], f32, tag="m1")
            m2 = p2.tile([32, T], f32, tag="m2")
            s1 = p2.tile([32, T], f32, tag="s1")
            s2 = p2.tile([32, T], f32, tag="s2")
            tmin = p2.tile([8, T], f32, tag="tmin")
            nc.vector.memset(m1[:], -1e30)
            nc.vector.memset(m2[:], -1e30)
            nc.vector.tensor_copy(m1[0:8, :], lg[:])
            for st in (4, 2, 1):
                shuf = [(i ^ st) for i in range(32)]
                nc.vector.stream_shuffle(s1[:], m1[:], shuf)
                nc.vector.stream_shuffle(s2[:], m2[:], shuf)
                nc.vector.tensor_tensor(tmin[:], m1[0:8, :], s1[0:8, :], ALU.min)
                nc.vector.tensor_tensor(m1[0:8, :], m1[0:8, :], s1[0:8, :], ALU.max)
                nc.vector.tensor_tensor(m2[0:8, :], m2[0:8, :], s2[0:8, :], ALU.max)
                nc.vector.tensor_tensor(m2[0:8, :], m2[0:8, :], tmin[:], ALU.max)

            lsub = p2.tile([E, T], f32, tag="lsub")
            nc.vector.tensor_sub(lsub[:], lg[:], m1[0:8, :])
            p_t = p2.tile([E, T], f32, tag="p")
            nc.scalar.activation(p_t[:], lsub[:], AF.Exp)
            mask = p2.tile([E, T], f32, tag="mask")
            nc.vector.tensor_tensor(mask[:], lg[:], m2[0:8, :], ALU.is_ge)
            dgap = p2.tile([E, T], f32, tag="dgap")
            nc.vector.tensor_sub(dgap[:], m2[0:8, :], m1[0:8, :])
            dexp = p2.tile([E, T], f32, tag="dexp")
            nc.scalar.activation(dexp[:], dgap[:], AF.Exp)
            nc.vector.tensor_scalar(dexp[:], dexp[:], 1.0, None, ALU.add)
            rec = p2.tile([E, T], f32, tag="rec")
            nc.vector.reciprocal(rec[:], dexp[:])
            comb = p2.tile([E, T], f32, tag="comb")
            nc.vector.tensor_mul(comb[:], p_t[:], mask[:])
            nc.vector.tensor_mul(comb[:], comb[:], rec[:])

            iota8_t = p2.tile([E, T], f32, tag="iota8")
            nc.sync.dma_start(out=iota8_t[:], in_=iota8_d[:])
            mid = p2.tile([E, T], f32, tag="mid")
            nc.vector.tensor_scalar(mid[:], iota8_t[:], 1.0, None, ALU.add)
            nc.vector.tensor_mul(mid[:], mid[:], mask[:])
            nc.vector.tensor_scalar(mid[:], mid[:], 1.0, None, ALU.subtract)
            nc.sync.dma_start(out=mid_d[:], in_=mid[:])

            # combine -> wcomb rows [t, 0:8]; zero-fill sentinel rows
            zf = p2.tile([128, 128], f32, tag="zf")
            nc.vector.memset(zf[:], 0.0)
            nc.sync.dma_start(out=wcomb_d[T:TPAD, :], in_=zf[:TPAD - T, :])
            for g in range(T // 128):
                cps = p2ps.tile([128, E], f32, tag="cps")
                nc.tensor.transpose(
                    cps[:], comb[:, g * 128:(g + 1) * 128], idf[:E, :E])
                csb = p2.tile([128, E], f32, tag="csb")
                nc.vector.tensor_copy(csb[:], cps[:])
                nc.sync.dma_start(
                    out=wcomb_d[g * 128:(g + 1) * 128, 0:8], in_=csb[:])

            # compact ids for ALL experts now (gpsimd is otherwise idle)
            for e in range(E):
                sgin = p2i.tile([16, FSG], f32, tag="sgin")
                nc.sync.dma_start(
                    out=sgin[:, :T // 16],
                    in_=mid_d.ap()[e].rearrange("(f p) -> p f", p=16))
                nc.vector.memset(sgin[:, T // 16:], float(SENT))
                sgout = p2i.tile([16, FSG], f32, tag="sgout")
                nf = p2i.tile([1, 1], u32, tag="nf")
                nc.gpsimd.sparse_gather(sgout[:], sgin[:], num_found=nf[:])
                idx16 = p2i.tile([16, C // 16], i16, tag="idx16")
                nc.vector.tensor_copy(idx16[:], sgout[:, :C // 16])
                for q in range(8):
                    nc.sync.dma_start(
                        out=idxr_all[q * 16:(q + 1) * 16, e, :], in_=idx16[:])

            # h rows to DRAM (PE transposes overlap the DVE routing chain)
            zh = p3.tile([128, D], bf16, tag="zh")
            nc.vector.memset(zh[:], 0.0)
            nc.sync.dma_start(out=h_dram[T:TPAD, :], in_=zh[:TPAD - T, :])
            for g in range(TPAD // 128):
                nc.sync.dma_start(
                    out=moe_d[g * 128:(g + 1) * 128, :], in_=zh[:])
        # ---------------- P4: expert FFNs on compacted tokens ----------------
        with tc.tile_pool(name="p4i", bufs=2) as p4i, \
             tc.tile_pool(name="p4g", bufs=2) as p4g, \
             tc.tile_pool(name="p4w", bufs=2) as p4w, \
             tc.tile_pool(name="p4he", bufs=2) as p4he, \
             tc.tile_pool(name="p4y", bufs=2) as p4y, \
             tc.tile_pool(name="p4ps1", bufs=2, space="PSUM") as ps1, \
             tc.tile_pool(name="p4ps2", bufs=3, space="PSUM") as ps2:
            g_tiles = {}

            def emit_gathers(e):
                ghT = p4g.tile([128, KD, C], bf16, tag="ghT")
                nc.gpsimd.dma_gather(
                    ghT[:], h_dram[:], idxr_all[:, e, :], C, C, D,
                    transpose=True)
                gw = p4g.tile([128, C5, 128], f32, tag="gw")
                nc.gpsimd.dma_gather(
                    gw[:], wcomb_d[:], idxr_all[:, e, :], C, C, 128,
                    transpose=False)
                g_tiles[e] = (ghT, gw)

            emit_gathers(0)
            for e in range(E):
                if e + 1 < E:
                    emit_gathers(e + 1)
                ghT, gw = g_tiles.pop(e)

                b1_t = p4i.tile([128, MH], f32, tag="b1")
                nc.sync.dma_start(out=b1_t[:], in_=b1_d[e])
                b2_t = p4i.tile([128, D], bf16, tag="b2")
                nc.sync.dma_start(out=b2_t[:], in_=b2_d[e])

                y_acc = p4y.tile([128, C5, D], f32, tag="y_acc")
                for hb in range(HB):
                    w1_blk = p4w.tile([128, KD, 1024], bf16, tag="w1_blk")
                    nc.sync.dma_start(
                        out=w1_blk[:],
                        in_=w1_d.ap()[e, :, hb * 1024:(hb + 1) * 1024]
                        .rearrange("(k p) m -> p k m", p=128))
                    w2_blk = p4w.tile([128, 8, D], bf16, tag="w2_blk")
                    nc.sync.dma_start(
                        out=w2_blk[:],
                        in_=w2_d.ap()[e, hb * 1024:(hb + 1) * 1024, :]
                        .rearrange("(k p) n -> p k n", p=128))

                    he_blk = p4he.tile([128, 8, C], bf16, tag="he_blk")
                    for m8 in range(8):
                        p1t = ps1.tile([128, C], f32, tag="p1t")
                        for ch0 in range(0, C, 512):
                            ch1 = min(ch0 + 512, C)
                            for k in range(KD):
                                nc.tensor.matmul(
                                    p1t[:, ch0:ch1],
                                    w1_blk[:, k, m8 * 128:(m8 + 1) * 128],
                                    ghT[:, k, ch0:ch1],
                                    start=(k == 0), stop=(k == KD - 1))
                        nc.scalar.activation(
                            he_blk[:, m8, :], p1t[:], AF.Relu,
                            bias=b1_t[:, hb * 8 + m8:hb * 8 + m8 + 1])

                    for c5 in range(C5):
                        for ch in range(D // 512):
                            p2t = ps2.tile([128, 512], f32, tag="p2t")
                            for k8 in range(8):
                                nc.tensor.matmul(
                                    p2t[:],
                                    he_blk[:, k8, c5 * 128:(c5 + 1) * 128],
                                    w2_blk[:, k8, ch * 512:(ch + 1) * 512],
                                    start=(k8 == 0), stop=(k8 == 7))
                            dst = y_acc[:, c5, ch * 512:(ch + 1) * 512]
                            if hb == 0:
                                nc.vector.tensor_copy(dst, p2t[:])
                            else:
                                nc.vector.tensor_add(dst, dst, p2t[:])

                ysb = p4y.tile([128, C5, D], bf16, tag="ysb")
                for c5 in range(C5):
                    nc.vector.tensor_add(
                        y_acc[:, c5, :], y_acc[:, c5, :], b2_t[:])
                    nc.vector.tensor_scalar(
                        ysb[:, c5, :], y_acc[:, c5, :],
                        gw[:, c5, e:e + 1], None, ALU.mult)
                nc.gpsimd.dma_scatter_add(
                    moe_d[:], ysb[:], idxr_all[:, e, :], C, C, D)

        # ---------------- P5: moe reload + transpose ----------------
        late = octx.enter_context(tc.tile_pool(name="late", bufs=1))
        with tc.tile_pool(name="p5i", bufs=1) as p5i:
            moeT_chunks = [
                late.tile([128, KD, 512], bf16, name=f"moeT{gch}", tag=f"moeT{gch}")
                for gch in range(T // 512)]
            idx_id = p5i.tile([128, T // 16], i16, tag="idx_id")
            nc.sync.dma_start(out=idx_id[:], in_=idx_id_d[:])
            for gch in range(T // 512):
                nc.gpsimd.dma_gather(
                    moeT_chunks[gch][:], moe_d[:],
                    idx_id[:, gch * 32:(gch + 1) * 32], 512, 512, D,
                    transpose=True)

        # ---------------- P6: head ----------------
        with tc.tile_pool(name="p6w", bufs=2) as p6w, \
             tc.tile_pool(name="p6o", bufs=3) as p6o, \
             tc.tile_pool(name="p6ps", bufs=3, space="PSUM") as p6ps:
            OBLK = 1024 if OUT >= 1024 else OUT
            for mb in range(OUT // OBLK):
                wh_blk = p6w.tile([128, KD, OBLK], bf16, tag="wh_blk")
                nc.sync.dma_start(
                    out=wh_blk[:],
                    in_=w_head_d.ap()[:, mb * OBLK:(mb + 1) * OBLK]
                    .rearrange("(k p) m -> p k m", p=128))
                for m8 in range(OBLK // 128):
                    orow = p6o.tile([128, T], f32, tag="orow")
                    for ch in range(T // 512):
                        pht = p6ps.tile([128, 512], f32, tag="pht")
                        for k in range(KD):
                            nc.tensor.matmul(
                                pht[:],
                                wh_blk[:, k, m8 * 128:(m8 + 1) * 128],
                                moeT_chunks[ch][:, k, :],
                                start=(k == 0), stop=(k == KD - 1))
                        nc.vector.tensor_copy(
                            orow[:, ch * 512:(ch + 1) * 512], pht[:])
                    r0 = mb * OBLK + m8 * 128
                    nc.sync.dma_start(out=outT_d[r0:r0 + 128, :], in_=orow[:])

    nc.compile()
    return nc


_NC_CACHE = None


def get_program():
    global _NC_CACHE
    if _NC_CACHE is None:
        _NC_CACHE = build_program()
    return _NC_CACHE


def prep_in_maps(x, W_in, b_in, W_gate, W1, b1, W2, b2, W_head):
    bf = ml_dtypes.bfloat16
    w_in_h = np.ascontiguousarray(W_in.astype(bf))
    b_in_h = np.ascontiguousarray(np.broadcast_to(
        b_in.astype(np.float32)[None, :], (128, D)))
    wg_eff_h = np.ascontiguousarray(
        W_in.astype(np.float32) @ W_gate.astype(np.float32))
    bg_eff_h = np.ascontiguousarray(
        (b_in.astype(np.float32) @ W_gate.astype(np.float32)).reshape(E, 1))
    w1_h = np.ascontiguousarray(W1.astype(bf))
    b1_h = np.ascontiguousarray(
        np.transpose(b1.astype(np.float32).reshape(E, MH, 128), (0, 2, 1)))
    w2_h = np.ascontiguousarray(W2.astype(bf))
    b2_h = np.ascontiguousarray(
        np.broadcast_to(b2.astype(bf)[:, None, :], (E, 128, D)))
    w_head_h = np.ascontiguousarray(W_head.astype(bf))
    xT = np.ascontiguousarray(x.astype(np.float32).T)

    in_maps = []
    for c in range(N_CORES):
        in_maps.append({
            "xT": np.ascontiguousarray(xT[:, c * T:(c + 1) * T]),
            "w_in": w_in_h,
            "b_in_bc": b_in_h,
            "wg_eff": wg_eff_h,
            "bg_eff": bg_eff_h,
            "w1": w1_h,
            "b1_c": b1_h,
            "w2": w2_h,
            "b2_r": b2_h,
            "w_head": w_head_h,
        })

    return in_maps


def kernel(**inputs):
    from concourse.bass_utils import run_bass_kernel_spmd

    in_maps = prep_in_maps(**inputs)
    nc = get_program()
    res = run_bass_kernel_spmd(nc, in_maps, list(range(N_CORES)))
    out = np.empty((N, OUT), dtype=np.float32)
    for c in range(N_CORES):
        out[c * T:(c + 1) * T, :] = res.results[c]["outT"].T
    return out


# revision 14
# speedup vs baseline: 1.0599x; 1.0284x over previous
"""MoE routing kernel for Trainium2, 8 NeuronCores, token-parallel.

Problem (nn_Network_2121713845020):
  h = x @ W_in + b_in                        [N, D]
  probs = softmax(h @ W_gate); top-2 renormalized combine weights
  moe = sum_e combine[:, e] * (relu(h @ W1[e] + b1[e]) @ W2[e] + b2[e])
  out = moe @ W_head                         [N, OUT]

Strategy: shard tokens across 8 cores (N/8 = 2048 each); every core holds
all expert weights, computes routing on-device in fp32, compacts per-expert
token ids with gpsimd sparse_gather, gathers assigned token rows with
dma_gather (capacity C=640 vs expected 512), runs the expert FFN in bf16
with fp32 accumulation, scales by gathered combine weights and
dma_scatter_adds back, then applies the head. Device returns out^T per
core; the host transposes and concatenates. No collectives.
"""

import os
import sys

sys.path.insert(0, "/opt/trn_rl_repo")

from contextlib import ExitStack

import numpy as np
import ml_dtypes

import concourse.bacc as bacc
import concourse.bass as bass
import concourse.mybir as mybir
import concourse.tile as tile

f32 = mybir.dt.float32
bf16 = mybir.dt.bfloat16
i16 = mybir.dt.int16
u32 = mybir.dt.uint32
AF = mybir.ActivationFunctionType
ALU = mybir.AluOpType

N_CORES = 8

if os.environ.get("MOE_SMALL"):
    N, D, H, E, OUT, C = 4096, 512, 1024, 8, 512, 256
else:
    N, D, H, E, OUT, C = 16384, 1024, 4096, 8, 4096, 640

T = N // N_CORES            # tokens per core
TPAD = T + 128              # +sentinel row space
SENT = T                    # sentinel token id (garbage row)
KD = D // 128               # K-tiles over D
MH = H // 128               # M-tiles over H
HB = H // 1024              # H blocks of 1024 (8 m-tiles each)
C5 = C // 128               # compact-token tiles
NCH = T // 512              # N chunks of 512 over tokens
FSG = T // 16 + C // 16     # sparse_gather input free size


def build_program():
    nc = bacc.Bacc("TRN2", target_bir_lowering=False, debug=False,
                   num_devices=N_CORES)

    xT_d = nc.dram_tensor("xT", [D, T], f32, kind="ExternalInput")
    w_in_d = nc.dram_tensor("w_in", [D, D], bf16, kind="ExternalInput")
    b_in_d = nc.dram_tensor("b_in_bc", [128, D], f32, kind="ExternalInput")
    wg_eff_d = nc.dram_tensor("wg_eff", [D, E], f32, kind="ExternalInput")
    bg_eff_d = nc.dram_tensor("bg_eff", [E, 1], f32, kind="ExternalInput")
    w1_d = nc.dram_tensor("w1", [E, D, H], bf16, kind="ExternalInput")
    b1_d = nc.dram_tensor("b1_c", [E, 128, MH], f32, kind="ExternalInput")
    w2_d = nc.dram_tensor("w2", [E, H, D], bf16, kind="ExternalInput")
    b2_d = nc.dram_tensor("b2_r", [E, 128, D], bf16, kind="ExternalInput")
    w_head_d = nc.dram_tensor("w_head", [D, OUT], bf16, kind="ExternalInput")
    outT_d = nc.dram_tensor("outT", [OUT, T], f32, kind="ExternalOutput")

    h_dram = nc.dram_tensor("h_scr", [TPAD, D], bf16)
    wcomb_d = nc.dram_tensor("wcomb_scr", [TPAD, 128], f32)
    mid_d = nc.dram_tensor("mid_scr", [E, T], f32)
    moe_d = nc.dram_tensor("moe_scr", [TPAD, D], bf16)

    idf_np = np.eye(128, dtype=np.float32)
    idf_d = nc.inline_tensor(np.ascontiguousarray(idf_np), name="id_f32")
    iota8_np = np.tile(np.arange(T, dtype=np.float32)[None, :], (E, 1))
    iota8_d = nc.inline_tensor(np.ascontiguousarray(iota8_np), name="iota8")
    idx_id_np = np.zeros((128, T // 16), dtype=np.int16)
    for j in range(T):
        for q in range(8):
            idx_id_np[q * 16 + j % 16, j // 16] = j
    idx_id_d = nc.inline_tensor(np.ascontiguousarray(idx_id_np), name="idx_id")

    TCH = T // 512  # routing chunks

    with tile.TileContext(nc) as tc, ExitStack() as octx:
        const = octx.enter_context(tc.tile_pool(name="const", bufs=1))
        idf = const.tile([128, 128], f32, tag="idf")
        nc.sync.dma_start(out=idf[:], in_=idf_d[:])

        persist = octx.enter_context(tc.tile_pool(name="persist", bufs=1))
        idxr_all = persist.tile([128, E, C // 16], i16, tag="idxr_all")
        OBLK = 1024 if OUT >= 1024 else OUT
        wh0 = persist.tile([128, KD, OBLK], bf16, tag="wh0")
        nc.sync.dma_start(
            out=wh0[:],
            in_=w_head_d.ap()[:, 0:OBLK].rearrange("(k p) m -> p k m", p=128))

        # ============ P1+P2: h, fp32 logits, routing, compaction ============
        with tc.tile_pool(name="p1c", bufs=1) as p1c, \
             tc.tile_pool(name="p1s", bufs=3) as p1s, \
             tc.tile_pool(name="p1r", bufs=4) as p1r, \
             tc.tile_pool(name="p2r", bufs=2) as p2r, \
             tc.tile_pool(name="p2i", bufs=2) as p2i, \
             tc.tile_pool(name="p1ps", bufs=2, space="PSUM") as p1ps, \
             tc.tile_pool(name="p2ps", bufs=2, space="PSUM") as p2ps, \
             tc.tile_pool(name="p1lg", bufs=1, space="PSUM") as p1lg:
            xT_t = p1c.tile([128, KD, T], bf16, tag="xT")
            w_in_t = p1c.tile([128, KD, D], bf16, tag="w_in")
            wr = w_in_d.ap().rearrange("(k p) m -> p k m", p=128)
            for k in range(KD):
                nc.sync.dma_start(out=w_in_t[:, k, :], in_=wr[:, k, :])
            wg_t = p1c.tile([128, KD, E], f32, tag="wg_eff")
            nc.sync.dma_start(
                out=wg_t[:], in_=wg_eff_d.ap().rearrange("(k p) e -> p k e", p=128))
            bg_t = p1c.tile([E, 1], f32, tag="bg_eff")
            nc.sync.dma_start(out=bg_t[:], in_=bg_eff_d[:])
            b_in_t = p1c.tile([128, D], f32, tag="b_in")
            nc.sync.dma_start(out=b_in_t[:], in_=b_in_d[:])
            lg = p1c.tile([E, T], f32, tag="lg")
            iota8_t = p1c.tile([E, T], f32, tag="iota8")
            nc.sync.dma_start(out=iota8_t[:], in_=iota8_d[:])

            # fp32 logits, streaming xT fp32 per k-tile (cast to bf16 too)
            lg_ps = p1lg.tile([E, T], f32, tag="lg_ps")
            xr = xT_d.ap().rearrange("(k p) t -> p k t", p=128)
            for k in range(KD):
                xtf = p1s.tile([128, T], f32, tag="xtf")
                nc.sync.dma_start(out=xtf[:], in_=xr[:, k, :])
                nc.vector.tensor_copy(xT_t[:, k, :], xtf[:])
                for ch in range(TCH):
                    sl = slice(ch * 512, (ch + 1) * 512)
                    nc.tensor.matmul(
                        lg_ps[:, sl], wg_t[:, k, :], xtf[:, sl],
                        start=(k == 0), stop=(k == KD - 1))
            nc.vector.tensor_scalar(lg[:], lg_ps[:], bg_t[:], None, ALU.add)

            # zero fills (sentinel rows, moe accumulator, wcomb tail)
            zh = p1r.tile([128, D], bf16, tag="zh")
            nc.vector.memset(zh[:], 0.0)
            nc.sync.dma_start(out=h_dram[T:TPAD, :], in_=zh[:TPAD - T, :])
            for g in range(TPAD // 128):
                nc.sync.dma_start(out=moe_d[g * 128:(g + 1) * 128, :], in_=zh[:])
            zf = p1r.tile([128, 128], f32, tag="zf")
            nc.vector.memset(zf[:], 0.0)
            nc.sync.dma_start(out=wcomb_d[T:TPAD, :], in_=zf[:TPAD - T, :])

            # routing in 512-token chunks (overlaps the h matmuls on PE)
            for tch in range(TCH):
                sl = slice(tch * 512, (tch + 1) * 512)
                m1 = p2r.tile([32, 512], f32, tag="m1")
                m2 = p2r.tile([32, 512], f32, tag="m2")
                s1 = p2r.tile([32, 512], f32, tag="s1")
                s2 = p2r.tile([32, 512], f32, tag="s2")
                tmin = p2r.tile([8, 512], f32, tag="tmin")
                nc.vector.memset(m1[:], -1e30)
                nc.vector.memset(m2[:], -1e30)
                nc.vector.tensor_copy(m1[0:8, :], lg[:, sl])
                for st in (4, 2, 1):
                    shuf = [(i ^ st) for i in range(32)]
                    nc.vector.stream_shuffle(s1[:], m1[:], shuf)
                    nc.vector.stream_shuffle(s2[:], m2[:], shuf)
                    nc.vector.tensor_tensor(tmin[:], m1[0:8, :], s1[0:8, :], ALU.min)
                    nc.vector.tensor_tensor(m1[0:8, :], m1[0:8, :], s1[0:8, :], ALU.max)
                    nc.vector.tensor_tensor(m2[0:8, :], m2[0:8, :], s2[0:8, :], ALU.max)
                    nc.vector.tensor_tensor(m2[0:8, :], m2[0:8, :], tmin[:], ALU.max)

                p_t = p2r.tile([E, 512], f32, tag="p")
                nc.vector.tensor_sub(p_t[:], lg[:, sl], m1[0:8, :])
                nc.scalar.activation(p_t[:], p_t[:], AF.Exp)
                mask = p2r.tile([E, 512], f32, tag="mask")
                nc.vector.tensor_tensor(mask[:], lg[:, sl], m2[0:8, :], ALU.is_ge)
                rec = p2r.tile([E, 512], f32, tag="rec")
                nc.vector.tensor_sub(rec[:], m2[0:8, :], m1[0:8, :])
                nc.scalar.activation(rec[:], rec[:], AF.Exp)
                nc.vector.tensor_scalar(rec[:], rec[:], 1.0, None, ALU.add)
                nc.vector.reciprocal(rec[:], rec[:])
                comb = p2r.tile([E, 512], f32, tag="comb")
                nc.vector.tensor_mul(comb[:], p_t[:], mask[:])
                nc.vector.tensor_mul(comb[:], comb[:], rec[:])

                mid = p2r.tile([E, 512], f32, tag="mid")
                nc.vector.tensor_scalar(
                    mid[:], iota8_t[:, sl], 1.0, None, ALU.add)
                nc.vector.tensor_mul(mid[:], mid[:], mask[:])
                nc.vector.tensor_scalar(mid[:], mid[:], 1.0, None, ALU.subtract)
                nc.sync.dma_start(out=mid_d[:, sl], in_=mid[:])

                for gq in range(4):
                    g = tch * 4 + gq
                    cps = p2ps.tile([128, E], f32, tag="cps")
                    nc.tensor.transpose(
                        cps[:], comb[:, gq * 128:(gq + 1) * 128], idf[:E, :E])
                    csb = p2i.tile([128, E], f32, tag="csb", bufs=4)
                    nc.vector.tensor_copy(csb[:], cps[:])
                    nc.sync.dma_start(
                        out=wcomb_d[g * 128:(g + 1) * 128, 0:8], in_=csb[:])

            # compact ids for ALL experts (gpsimd)
            for e in range(E):
                sgin = p2i.tile([16, FSG], f32, tag="sgin")
                nc.sync.dma_start(
                    out=sgin[:, :T // 16],
                    in_=mid_d.ap()[e].rearrange("(f p) -> p f", p=16))
                nc.vector.memset(sgin[:, T // 16:], float(SENT))
                sgout = p2i.tile([16, FSG], f32, tag="sgout")
                nf = p2i.tile([1, 1], u32, tag="nf")
                nc.gpsimd.sparse_gather(sgout[:], sgin[:], num_found=nf[:])
                idx16 = p2i.tile([16, C // 16], i16, tag="idx16")
                nc.vector.tensor_copy(idx16[:], sgout[:, :C // 16])
                for q in range(8):
                    nc.sync.dma_start(
                        out=idxr_all[q * 16:(q + 1) * 16, e, :], in_=idx16[:])

            # h rows (token-major), straight to DRAM
            for g in range(T // 128):
                hrow = p1r.tile([128, D], bf16, tag="hrow")
                for ch in range(D // 512):
                    hps = p1ps.tile([128, 512], f32, tag="hps")
                    for k in range(KD):
                        nc.tensor.matmul(
                            hps[:],
                            xT_t[:, k, g * 128:(g + 1) * 128],
                            w_in_t[:, k, ch * 512:(ch + 1) * 512],
                            start=(k == 0), stop=(k == KD - 1))
                    nc.vector.tensor_add(
                        hrow[:, ch * 512:(ch + 1) * 512], hps[:],
                        b_in_t[:, ch * 512:(ch + 1) * 512])
                nc.sync.dma_start(
                    out=h_dram[g * 128:(g + 1) * 128, :], in_=hrow[:])

        # ---------------- P4: expert FFNs on compacted tokens ----------------
        with tc.tile_pool(name="p4i", bufs=2) as p4i, \
             tc.tile_pool(name="p4g", bufs=2) as p4g, \
             tc.tile_pool(name="p4w", bufs=2) as p4w, \
             tc.tile_pool(name="p4he", bufs=2) as p4he, \
             tc.tile_pool(name="p4y", bufs=1) as p4y, \
             tc.tile_pool(name="p4ys", bufs=2) as p4ys, \
             tc.tile_pool(name="p4ps1", bufs=2, space="PSUM") as ps1, \
             tc.tile_pool(name="p4ps2", bufs=3, space="PSUM") as ps2:
            g_tiles = {}

            def emit_gathers(e):
                ghT = p4g.tile([128, KD, C], bf16, tag="ghT")
                nc.gpsimd.dma_gather(
                    ghT[:], h_dram[:], idxr_all[:, e, :], C, C, D,
                    transpose=True)
                gw = p4g.tile([128, C5, 128], f32, tag="gw")
                nc.gpsimd.dma_gather(
                    gw[:], wcomb_d[:], idxr_all[:, e, :], C, C, 128,
                    transpose=False)
                g_tiles[e] = (ghT, gw)

            emit_gathers(0)
            for e in range(E):
                if e + 1 < E:
                    emit_gathers(e + 1)
                ghT, gw = g_tiles.pop(e)

                b1_t = p4i.tile([128, MH], f32, tag="b1")
                nc.sync.dma_start(out=b1_t[:], in_=b1_d[e])
                b2_t = p4i.tile([128, D], bf16, tag="b2")
                nc.sync.dma_start(out=b2_t[:], in_=b2_d[e])

                y_acc = p4y.tile([128, C5, D], f32, tag="y_acc")
                for hb in range(HB):
                    w1_blk = p4w.tile([128, KD, 1024], bf16, tag="w1_blk")
                    nc.sync.dma_start(
                        out=w1_blk[:],
                        in_=w1_d.ap()[e, :, hb * 1024:(hb + 1) * 1024]
                        .rearrange("(k p) m -> p k m", p=128))
                    w2_blk = p4w.tile([128, 8, D], bf16, tag="w2_blk")
                    nc.sync.dma_start(
                        out=w2_blk[:],
                        in_=w2_d.ap()[e, hb * 1024:(hb + 1) * 1024, :]
                        .rearrange("(k p) n -> p k n", p=128))

                    he_blk = p4he.tile([128, 8, C], bf16, tag="he_blk")
                    for m8 in range(8):
                        p1t = ps1.tile([128, C], f32, tag="p1t")
                        for ch0 in range(0, C, 512):
                            ch1 = min(ch0 + 512, C)
                            for k in range(KD):
                                nc.tensor.matmul(
                                    p1t[:, ch0:ch1],
                                    w1_blk[:, k, m8 * 128:(m8 + 1) * 128],
                                    ghT[:, k, ch0:ch1],
                                    start=(k == 0), stop=(k == KD - 1))
                        nc.scalar.activation(
                            he_blk[:, m8, :], p1t[:], AF.Relu,
                            bias=b1_t[:, hb * 8 + m8:hb * 8 + m8 + 1])

                    for c5 in range(C5):
                        for ch in range(D // 512):
                            p2t = ps2.tile([128, 512], f32, tag="p2t")
                            for k8 in range(8):
                                nc.tensor.matmul(
                                    p2t[:],
                                    he_blk[:, k8, c5 * 128:(c5 + 1) * 128],
                                    w2_blk[:, k8, ch * 512:(ch + 1) * 512],
                                    start=(k8 == 0), stop=(k8 == 7))
                            dst = y_acc[:, c5, ch * 512:(ch + 1) * 512]
                            if hb == 0:
                                nc.vector.tensor_copy(dst, p2t[:])
                            else:
                                nc.vector.tensor_add(dst, dst, p2t[:])

                ysb = p4ys.tile([128, C5, D], bf16, tag="ysb")
                for c5 in range(C5):
                    nc.vector.tensor_add(
                        y_acc[:, c5, :], y_acc[:, c5, :], b2_t[:])
                    nc.vector.tensor_scalar(
                        ysb[:, c5, :], y_acc[:, c5, :],
                        gw[:, c5, e:e + 1], None, ALU.mult)
                nc.gpsimd.dma_scatter_add(
                    moe_d[:], ysb[:], idxr_all[:, e, :], C, C, D)

        # ---------------- P5+P6: moe gather-transpose + head ----------------
        with tc.tile_pool(name="p5i", bufs=1) as p5i, \
             tc.tile_pool(name="p6w", bufs=2) as p6w, \
             tc.tile_pool(name="p6o", bufs=3) as p6o, \
             tc.tile_pool(name="p6ps", bufs=3, space="PSUM") as p6ps:
            moeT_chunks = [
                p5i.tile([128, KD, 512], bf16, name=f"moeT{gch}", tag=f"moeT{gch}")
                for gch in range(T // 512)]
            idx_id = p5i.tile([128, T // 16], i16, tag="idx_id")
            nc.sync.dma_start(out=idx_id[:], in_=idx_id_d[:])
            for gch in range(T // 512):
                nc.gpsimd.dma_gather(
                    moeT_chunks[gch][:], moe_d[:],
                    idx_id[:, gch * 32:(gch + 1) * 32], 512, 512, D,
                    transpose=True)

            for mb in range(OUT // OBLK):
                if mb == 0:
                    wh_blk = wh0
                else:
                    wh_blk = p6w.tile([128, KD, OBLK], bf16, tag="wh_blk")
                    nc.sync.dma_start(
                        out=wh_blk[:],
                        in_=w_head_d.ap()[:, mb * OBLK:(mb + 1) * OBLK]
                        .rearrange("(k p) m -> p k m", p=128))
                for m8 in range(OBLK // 128):
                    orow = p6o.tile([128, T], f32, tag="orow")
                    for ch in range(T // 512):
                        pht = p6ps.tile([128, 512], f32, tag="pht")
                        for k in range(KD):
                            nc.tensor.matmul(
                                pht[:],
                                wh_blk[:, k, m8 * 128:(m8 + 1) * 128],
                                moeT_chunks[ch][:, k, :],
                                start=(k == 0), stop=(k == KD - 1))
                        nc.vector.tensor_copy(
                            orow[:, ch * 512:(ch + 1) * 512], pht[:])
                    r0 = mb * OBLK + m8 * 128
                    nc.sync.dma_start(out=outT_d[r0:r0 + 128, :], in_=orow[:])

    nc.compile()
    return nc


_NC_CACHE = None


def get_program():
    global _NC_CACHE
    if _NC_CACHE is None:
        _NC_CACHE = build_program()
    return _NC_CACHE


def prep_in_maps(x, W_in, b_in, W_gate, W1, b1, W2, b2, W_head):
    bf = ml_dtypes.bfloat16
    w_in_h = np.ascontiguousarray(W_in.astype(bf))
    b_in_h = np.ascontiguousarray(np.broadcast_to(
        b_in.astype(np.float32)[None, :], (128, D)))
    wg_eff_h = np.ascontiguousarray(
        W_in.astype(np.float32) @ W_gate.astype(np.float32))
    bg_eff_h = np.ascontiguousarray(
        (b_in.astype(np.float32) @ W_gate.astype(np.float32)).reshape(E, 1))
    w1_h = np.ascontiguousarray(W1.astype(bf))
    b1_h = np.ascontiguousarray(
        np.transpose(b1.astype(np.float32).reshape(E, MH, 128), (0, 2, 1)))
    w2_h = np.ascontiguousarray(W2.astype(bf))
    b2_h = np.ascontiguousarray(
        np.broadcast_to(b2.astype(bf)[:, None, :], (E, 128, D)))
    w_head_h = np.ascontiguousarray(W_head.astype(bf))
    xT = np.ascontiguousarray(x.astype(np.float32).T)

    in_maps = []
    for c in range(N_CORES):
        in_maps.append({
            "xT": np.ascontiguousarray(xT[:, c * T:(c + 1) * T]),
            "w_in": w_in_h,
            "b_in_bc": b_in_h,
            "wg_eff": wg_eff_h,
            "bg_eff": bg_eff_h,
            "w1": w1_h,
            "b1_c": b1_h,
            "w2": w2_h,
            "b2_r": b2_h,
            "w_head": w_head_h,
        })

    return in_maps


def kernel(**inputs):
    from concourse.bass_utils import run_bass_kernel_spmd

    in_maps = prep_in_maps(**inputs)
    nc = get_program()
    res = run_bass_kernel_spmd(nc, in_maps, list(range(N_CORES)))
    out = np.empty((N, OUT), dtype=np.float32)
    for c in range(N_CORES):
        out[c * T:(c + 1) * T, :] = res.results[c]["outT"].T
    return out
